# revision 1
# baseline (speedup 1.0000x reference)
"""CTC batch cost (Keras convention) on 8 Trainium2 NeuronCores.

Raw-Bass static pipeline (no Tile): explicit engine streams + semaphores.
Raw mode emits semaphore waits as standalone sequencer instructions, which
avoids the 1-wait limit of embedded sync on matmul/DMA pseudo-instructions.

Per core (32 batch rows):
  - Host uploads log(y_pred+1e-7) packed with one-hot gather matrices
    [b, C, T+S], skewed transition masks, and a +32 partition permutation.
  - Gather: PE one-hot matmuls produce logP [S, T] per b (exact gather);
    ScalarE copies PSUM->SBUF; DMAs scatter into a skewed slab with
    partitions = (b, time-segment j), free dim = wavefront cells.
  - Viterbi pass (log space, overflow-immune): 100-cycle wavefront, per
    cycle one scalar_tensor_tensor (add/max) + one tensor_tensor_scan
    (max, add) on DVE; cross-segment halos via PE permutation matmul +
    ScalarE copies.
  - Per-segment max-path levels via strided max-reduces -> per-partition
    exp biases (measured rates + compile-time khat tilt).
  - ScalarE exp -> scaled linear slab; forward pass = same wavefront with
    (mult/add) + scan (add, mult); state bounded within ~e+-50.
  - loss = -(log(alpha_T[S-1]+alpha_T[S-2]) + Vstar_T + 128*sum(khat)).

The program is input-value-independent; built/compiled once, reused.
"""

from contextlib import ExitStack

import numpy as np

import concourse.bass as bass
import concourse.mybir as mybir
from concourse.bass_utils import run_bass_kernel_spmd

F32 = mybir.dt.float32
AF = mybir.ActivationFunctionType
OP = mybir.AluOpType
NEG = -1e30
EPS = 1e-7

B, T, C, U = 256, 512, 128, 48
S = 2 * U + 1          # 97
BLANK = C - 1
NCORES = 8
BPC = B // NCORES      # 32
NSEG = 4
SEG = T // NSEG        # 128
W = SEG + 1            # cell width (halo slot + 128 values)
NCYC = S + NSEG - 1    # 100
LEAD = 2
KHAT = (0.252, 0.137, 0.137, 0.137)
KSUM = SEG * sum(KHAT)
GRP = 8                # b per mega-DMA
NGRP = BPC // GRP      # 4
PSLAB = NCYC * SEG     # 12800
VSLAB = (NCYC + LEAD) * W

_cache = {}


def _cb(s0):
    return (s0 + LEAD) * W


def build_program():
    nc = bass.Bass()
    ygpack = nc.declare_dram_parameter("ygpack", [BPC, C, T + S], F32, isOutput=False)
    mlog = nc.declare_dram_parameter("mlog", [128, NCYC], F32, isOutput=False)
    mlin = nc.declare_dram_parameter("mlin", [128, NCYC], F32, isOutput=False)
    perm = nc.declare_dram_parameter("perm", [128, 128], F32, isOutput=False)
    paug = nc.declare_dram_parameter("paug", [128, 128], F32, isOutput=False)
    negc = nc.declare_dram_parameter("negc", [128, 1], F32, isOutput=False)
    loss = nc.declare_dram_parameter("loss", [BPC, 1], F32, isOutput=True)

    ctx = ExitStack()

    def sbuf(shape, name):
        return ctx.enter_context(nc.sbuf_tensor(name, shape, F32))

    def psumt(shape, name):
        return ctx.enter_context(nc.psum_tensor(name, shape, F32))

    def semp(name):
        return ctx.enter_context(nc.semaphore(name))

    with ctx:
        permst = sbuf([128, 128], "permst")
        paugt = sbuf([128, 128], "paugt")
        negct = sbuf([128, 1], "negct")
        mlogt = sbuf([128, NCYC], "mlogt")
        mlint = sbuf([128, NCYC], "mlint")
        ygt = [sbuf([C, GRP * (T + S)], f"ygt{i}") for i in range(2)]
        stg = [sbuf([S, T], f"stg{i}") for i in range(4)]
        pslab = sbuf([128, PSLAB], "pslab")
        phslab = sbuf([128, PSLAB], "phslab")
        vslab = sbuf([128, VSLAB], "vslab")
        uu = [sbuf([128, SEG], f"u{i}") for i in range(2)]
        atile = sbuf([128, 1], "atile")
        ctile = sbuf([128, 1], "ctile")
        btile = sbuf([128, 1], "btile")
        khat_t = sbuf([128, 1], "khat_t")
        d1 = sbuf([128, 1], "d1")
        bias_t = sbuf([128, 1], "bias_t")
        rout = [sbuf([128, 1], f"rout{j}") for j in range(NSEG)]
        vt = sbuf([128, 1], "vt")
        lt = sbuf([128, 1], "lt")
        st = sbuf([128, 1], "st")
        lossT = sbuf([128, 1], "lossT")

        ps = [psumt([S, T], f"ps{i}") for i in range(2)]
        ph = [psumt([128, 1], f"ph{i}") for i in range(2)]
        bps = psumt([128, 1], "bps")

        sem_c = semp("sem_c")
        sem_y = [semp("sem_y0"), semp("sem_y1")]
        sem_sk = [semp(f"sem_sk{i}") for i in range(4)]  # per stg-slot skews
        sem_v = semp("sem_v")
        sem_a = semp("sem_a")
        sem_p = semp("sem_p")
        sem_o = semp("sem_o")

        # ---- planned semaphore tick values ----
        # PE: 32 gather mms (1..32), viterbi perms (33..131), btile perm
        # (132), linear perms (133..231)
        p_mm = {b: b + 1 for b in range(BPC)}
        p_perm_v = {s0: BPC + 1 + s0 for s0 in range(NCYC - 1)}
        p_bperm = BPC + NCYC
        p_perm_l = {s0: p_bperm + 1 + s0 for s0 in range(NCYC - 1)}
        # ACT: stg copies (1..32), viterbi halos (33..131: one inc per
        # cycle after 2nd copy), atile/ctile copies (132..138), btile
        # (139), exp (140), linear halos (141..239), Ln (240), final (241)
        a_cp = {b: b + 1 for b in range(BPC)}
        a_hv = {s0: BPC + 1 + s0 for s0 in range(NCYC - 1)}
        a_abc = BPC + NCYC - 1 + 7
        a_btile = a_abc + 1
        a_exp = a_btile + 1
        a_hl = {s0: a_exp + 1 + s0 for s0 in range(NCYC - 1)}
        a_ln = a_exp + NCYC
        a_fin = a_ln + 1
        # DVE: 6 pslab fake memsets + 3 viterbi init (->9), viterbi scans
        # (10..109), 4 reduces (110..113), d1 (114), bias (115), linear
        # init (116..118), linear scans (119..218), vt (219), st (220)
        v_ms = 9
        v_scan_v = {s0: v_ms + 1 + s0 for s0 in range(NCYC)}
        v_red = {j: v_ms + NCYC + 1 + j for j in range(NSEG)}
        v_bias = v_ms + NCYC + NSEG + 2
        v_init_l = v_bias + 3
        v_scan_l = {s0: v_init_l + 1 + s0 for s0 in range(NCYC)}
        v_vt = v_init_l + NCYC + 1
        v_st = v_vt + 1

        with nc.Block() as block:

            @block.sync
            def _(sync):
                sync.dma_start(permst[:], perm[:]).then_inc(sem_c, 16)
                sync.dma_start(paugt[:], paug[:]).then_inc(sem_c, 16)
                sync.dma_start(negct[:], negc[:]).then_inc(sem_c, 16)
                sync.dma_start(mlogt[:], mlog[:]).then_inc(sem_c, 16)
                sync.dma_start(mlint[:], mlin[:]).then_inc(sem_c, 16)
                ygr = ygpack[:].rearrange("b c w -> c b w")
                p3 = pslab[:].rearrange("p (c w) -> p c w", w=SEG)

                def mega(g):
                    if g >= 2:
                        sync.wait_ge(sem_p, p_mm[(g - 1) * GRP - 1])
                    sync.dma_start(
                        ygt[g % 2][:].rearrange("c (b w) -> c b w", w=T + S),
                        ygr[:, g * GRP:(g + 1) * GRP, :],
                    ).then_inc(sem_y[g % 2], 16)

                def skews(b):
                    sync.wait_ge(sem_a, a_cp[b])
                    for j in range(NSEG):
                        p = b + 32 * j
                        dst = pslab[p:p + 1, j * SEG:j * SEG + S * SEG]
                        sync.dma_start(
                            dst, stg[b % 4][:, j * SEG:(j + 1) * SEG]
                        ).then_inc(sem_sk[b % 4], 16)

                mega(0)
                mega(1)
                sync.wait_ge(sem_v, 6)  # pslab fake-region memsets done
                for b in range(GRP):
                    skews(b)
                mega(2)
                for b in range(GRP, 2 * GRP):
                    skews(b)
                mega(3)
                for b in range(2 * GRP, BPC):
                    skews(b)
                sync.wait_ge(sem_a, a_fin)
                sync.dma_start(loss[:, :], lossT[96:128, :]).then_inc(sem_o, 16)
                sync.wait_ge(sem_o, 16)

            @block.tensor
            def _(tensor):
                for b in range(BPC):
                    g = b // GRP
                    if b % GRP == 0:
                        tensor.wait_ge(sem_y[g % 2], 16 * (g // 2 + 1))
                    if b >= 2:
                        tensor.wait_ge(sem_a, a_cp[b - 2])
                    yg3 = ygt[g % 2][:].rearrange("c (b w) -> c b w", w=T + S)
                    bl = b % GRP
                    nc.tensor.matmul(
                        ps[b % 2][:], lhsT=yg3[:, bl, T:T + S],
                        rhs=yg3[:, bl, 0:T], start=True, stop=True,
                    ).then_inc(sem_p, 1)

                def perms(v_scan, a_h, aug):
                    for s0 in range(NCYC - 1):
                        tensor.wait_ge(sem_v, v_scan[s0])
                        if s0 >= 2:
                            tensor.wait_ge(sem_a, a_h[s0 - 2])
                        if aug:
                            nc.tensor.matmul(
                                ph[s0 % 2][:], lhsT=permst[:],
                                rhs=vslab[:, _cb(s0) + SEG:_cb(s0) + SEG + 1],
                                start=True, stop=False,
                            )
                            nc.tensor.matmul(
                                ph[s0 % 2][:], lhsT=paugt[:], rhs=negct[:],
                                start=False, stop=True,
                            ).then_inc(sem_p, 1)
                        else:
                            nc.tensor.matmul(
                                ph[s0 % 2][:], lhsT=permst[:],
                                rhs=vslab[:, _cb(s0) + SEG:_cb(s0) + SEG + 1],
                                start=True, stop=True,
                            ).then_inc(sem_p, 1)

                tensor.wait_ge(sem_c, 80)
                perms(v_scan_v, a_hv, True)
                tensor.wait_ge(sem_a, a_abc)
                nc.tensor.matmul(bps[:], lhsT=permst[:], rhs=ctile[:],
                                 start=True, stop=True).then_inc(sem_p, 1)
                perms(v_scan_l, a_hl, False)

            @block.scalar
            def _(scalar):
                for b in range(BPC):
                    scalar.wait_ge(sem_p, p_mm[b])
                    if b >= 4:
                        # stg slot b%4 reused: b-4's skew DMAs must be done
                        scalar.wait_ge(sem_sk[b % 4], 16 * 4 * (b // 4))
                    nc.scalar.activation(out=stg[b % 4][:], in_=ps[b % 2][:],
                                         func=AF.Copy).then_inc(sem_a, 1)

                def halos(p_perm):
                    for s0 in range(NCYC - 1):
                        scalar.wait_ge(sem_p, p_perm[s0])
                        nc.scalar.activation(
                            out=vslab[32:64, _cb(s0 + 1):_cb(s0 + 1) + 1],
                            in_=ph[s0 % 2][32:64], func=AF.Copy)
                        nc.scalar.activation(
                            out=vslab[64:128, _cb(s0 + 1):_cb(s0 + 1) + 1],
                            in_=ph[s0 % 2][64:128], func=AF.Copy,
                        ).then_inc(sem_a, 1)

                halos(p_perm_v)
                for j in range(1, NSEG + 1):
                    scalar.wait_ge(sem_v, v_red[j - 1])
                    lo, hi = 32 * (j - 1), 32 * j
                    nc.scalar.activation(out=atile[lo:hi], in_=rout[j - 1][lo:hi],
                                         func=AF.Copy).then_inc(sem_a, 1)
                    if j < NSEG:
                        nc.scalar.activation(out=ctile[lo:hi],
                                             in_=rout[j - 1][lo:hi],
                                             func=AF.Copy).then_inc(sem_a, 1)
                scalar.wait_ge(sem_p, p_bperm)
                nc.scalar.activation(out=btile[:], in_=bps[:],
                                     func=AF.Copy).then_inc(sem_a, 1)
                scalar.wait_ge(sem_v, v_bias)
                for i in range(4):
                    scalar.wait_ge(sem_sk[i], 16 * 4 * (BPC // 4))
                nc.scalar.activation(out=phslab[:], in_=pslab[:], func=AF.Exp,
                                     bias=bias_t[:], scale=1.0).then_inc(sem_a, 1)
                halos(p_perm_l)
                scalar.wait_ge(sem_v, v_vt)
                nc.scalar.activation(out=lt[96:128], in_=vt[96:128],
                                     func=AF.Ln).then_inc(sem_a, 1)
                scalar.wait_ge(sem_v, v_st)
                nc.scalar.activation(out=lossT[96:128], in_=st[96:128],
                                     func=AF.Copy, scale=-1.0,
                                     bias=-KSUM).then_inc(sem_a, 1)

            @block.vector
            def _(vector):
                p3 = pslab[:].rearrange("p (c w) -> p c w", w=SEG)
                v3 = vslab[:].rearrange("p (c w) -> p c w", w=W)
                for j in range(NSEG):
                    if j > 0:
                        nc.vector.memset(p3[32 * j:32 * (j + 1), 0:j, :],
                                         NEG).then_inc(sem_v, 1)
                    if j < NSEG - 1:
                        nc.vector.memset(p3[32 * j:32 * (j + 1), j + S:NCYC, :],
                                         NEG).then_inc(sem_v, 1)

                def init_slab(viterbi, base):
                    z = NEG if viterbi else 0.0
                    nc.vector.memset(vslab[:, 0:LEAD * W], z).then_inc(sem_v, 1)
                    nc.vector.memset(v3[:, LEAD:, 0], z).then_inc(sem_v, 1)
                    vector.drain()
                    nc.vector.memset(vslab[0:32, _cb(0):_cb(0) + 1],
                                     0.0 if viterbi else 1.0).then_inc(sem_v, 1)

                def cycles(viterbi, data_slab, a_h, p_perm):
                    for s0 in range(NCYC):
                        if s0 >= 2:
                            vector.wait_ge(sem_a, a_h[s0 - 2])
                        vector.drain()
                        nc.vector.scalar_tensor_tensor(
                            out=uu[s0 % 2][:],
                            in0=vslab[:, _cb(s0 - 2):_cb(s0 - 2) + SEG],
                            scalar=(mlogt if viterbi else mlint)[:, s0:s0 + 1],
                            in1=vslab[:, _cb(s0 - 1):_cb(s0 - 1) + SEG],
                            op0=OP.add if viterbi else OP.mult,
                            op1=OP.max if viterbi else OP.add,
                        )
                        if s0 >= 1:
                            vector.wait_ge(sem_p, p_perm[s0 - 1])
                        vector.drain()
                        nc.vector.tensor_tensor_scan(
                            out=vslab[:, _cb(s0) + 1:_cb(s0) + 1 + SEG],
                            data0=uu[s0 % 2][:],
                            data1=data_slab[:, s0 * SEG:(s0 + 1) * SEG],
                            initial=(ph[(s0 - 1) % 2][:, 0:1] if s0 >= 1
                                     else vslab[:, _cb(s0):_cb(s0) + 1]),
                            op0=OP.max if viterbi else OP.add,
                            op1=OP.add if viterbi else OP.mult,
                        ).then_inc(sem_v, 1)

                init_slab(True, 6)
                for i in range(4):
                    vector.wait_ge(sem_sk[i], 16 * 4 * (BPC // 4))
                vector.wait_ge(sem_c, 80)
                cycles(True, pslab, a_hv, p_perm_v)
                vector.drain()
                nc.vector.memset(ctile[:], 0.0)
                for j in range(1, NSEG + 1):
                    nc.vector.tensor_reduce(
                        out=rout[j - 1][:],
                        in_=v3[:, (j - 1) + LEAD:(j - 1) + LEAD + S, SEG],
                        axis=mybir.AxisListType.X, op=OP.max,
                    ).then_inc(sem_v, 1)
                for j in range(NSEG):
                    nc.vector.memset(khat_t[32 * j:32 * (j + 1)], KHAT[j])
                vector.wait_ge(sem_a, a_btile)
                nc.vector.tensor_tensor(out=d1[:], in0=atile[:], in1=btile[:],
                                        op=OP.subtract).then_inc(sem_v, 1)
                vector.drain()
                nc.vector.scalar_tensor_tensor(
                    out=bias_t[:], in0=d1[:], scalar=-1.0 / SEG, in1=khat_t[:],
                    op0=OP.mult, op1=OP.subtract).then_inc(sem_v, 1)
                # linear init: wait until all viterbi-state consumers done
                vector.wait_ge(sem_a, a_exp)
                vector.wait_ge(sem_p, p_bperm)
                init_slab(False, 115)
                cycles(False, phslab, a_hl, p_perm_l)
                vector.drain()
                nc.vector.tensor_tensor(
                    out=vt[96:128],
                    in0=vslab[96:128, _cb(S + 1) + SEG:_cb(S + 1) + SEG + 1],
                    in1=vslab[96:128, _cb(S + 2) + SEG:_cb(S + 2) + SEG + 1],
                    op=OP.add).then_inc(sem_v, 1)
                vector.wait_ge(sem_a, a_ln)
                nc.vector.tensor_tensor(out=st[96:128], in0=lt[96:128],
                                        in1=atile[96:128],
                                        op=OP.add).then_inc(sem_v, 1)

    return nc


def host_prep(y_true, y_pred):
    y_true = np.asarray(y_true)
    y_pred = np.asarray(y_pred, dtype=np.float32)
    ext = np.full((B, S), BLANK, dtype=np.int64)
    ext[:, 1::2] = y_true.astype(np.int64)
    sh = np.concatenate([np.full((B, 2), -1, dtype=np.int64), ext[:, :-2]], axis=1)
    m = ((ext != BLANK) & (ext != sh))

    lq = np.log(y_pred + EPS).astype(np.float32)  # [B, T, C]

    in_maps = []
    for k in range(NCORES):
        bs = slice(k * BPC, (k + 1) * BPC)
        lqt = np.transpose(lq[bs], (0, 2, 1))  # [32, C, T]
        g = np.zeros((BPC, C, S), dtype=np.float32)
        eb = ext[bs]
        for b in range(BPC):
            g[b, eb[b], np.arange(S)] = 1.0
        ygp = np.ascontiguousarray(np.concatenate([lqt, g], axis=2))
        mk = m[bs]
        mlogv = np.full((128, NCYC), NEG, dtype=np.float32)
        mlinv = np.zeros((128, NCYC), dtype=np.float32)
        for j in range(NSEG):
            for s0 in range(NCYC):
                s = s0 - j
                if 0 <= s < S:
                    mlogv[32 * j:32 * (j + 1), s0] = np.where(mk[:, s], 0.0, NEG)
                    mlinv[32 * j:32 * (j + 1), s0] = mk[:, s].astype(np.float32)
        permv = np.zeros((128, 128), dtype=np.float32)
        for kk in range(96):
            permv[kk, kk + 32] = 1.0
        paugv = np.zeros((128, 128), dtype=np.float32)
        for kk in range(32):
            paugv[kk, kk] = 1.0
        negcv = np.full((128, 1), NEG, dtype=np.float32)
        in_maps.append({"ygpack": ygp, "mlog": mlogv, "mlin": mlinv,
                        "perm": permv, "paug": paugv, "negc": negcv})
    return in_maps


def _ensure_axon_devices():
    """Best-effort: make sure the axon PJRT devices are visible even if the
    calling process pinned jax_platforms to cpu (the reference needs cpu;
    run_bass_kernel_spmd needs the 8 NeuronCore devices)."""
    import jax
    try:
        devs = jax.devices()
        if len(devs) >= NCORES and all(d.platform != "cpu" for d in devs[:1]):
            return
    except Exception:
        pass
    try:
        jax.config.update("jax_platforms", None)
        jax.devices()
    except Exception:
        pass


def kernel(y_true, y_pred):
    _ensure_axon_devices()
    if "nc" not in _cache:
        _cache["nc"] = build_program()
    nc = _cache["nc"]
    in_maps = host_prep(y_true, y_pred)
    res = run_bass_kernel_spmd(nc, in_maps, list(range(NCORES)))
    out = np.concatenate([np.asarray(res.results[k]["loss"], dtype=np.float32)
                          for k in range(NCORES)], axis=0)
    return out.reshape(B, 1).astype(np.float32)



# revision 25
# speedup vs baseline: 14.6399x; 14.6399x over previous
"""CTC batch cost (Keras convention) on 8 Trainium2 NeuronCores.

Raw-Bass static pipeline (no Tile): explicit engine streams + semaphores.

Per core (32 batch rows), v2 pipeline:
  - Host uploads log(y_pred+1e-7) packed with one-hot gather matrices
    [b, C, T+S] in bf16, plus skewed transition masks and a +32 partition
    permutation.
  - Gather: PE one-hot bf16 matmuls produce logP [S, T] per b (exact
    gather of bf16-quantized logp); DVE copies PSUM->SBUF staging.
  - Skew transpose via DRAM round trip: per-b DMA (ACT queue) writes the
    staging tile into a DRAM scratch laid out as the final skewed slab
    image [128, NCYC*SEG] (partition = (b, time-segment j), free =
    wavefront cells); then 4 chunked DMAs (SP queue) bring the image into
    SBUF.  Output APs span many partitions, so the DMA-time cost model
    (free-bytes-per-partition) is ~128x cheaper than single-partition
    scatters.
  - Viterbi pass (log space, overflow-immune): 100-cycle wavefront, per
    cycle one scalar_tensor_tensor (add/max) + one tensor_tensor_scan
    (max, add) on DVE; cross-segment halos via a single PE permutation
    matmul into PSUM rows 32:128 (rows 0:32 preset per pass) + one
    ScalarE copy.  Slab chunks gate cycle groups so fill overlaps the
    recursion.
  - Per-segment max-path levels via strided max-reduces -> per-partition
    exp biases (measured rates + compile-time khat tilt).
  - ScalarE exp (4 chunks, overlapping the linear pass) -> scaled linear
    slab; forward pass = same wavefront with (mult/add) + scan (add,
    mult); state bounded within ~e+-50.
  - loss = -(log(alpha_T[S-1]+alpha_T[S-2]) + Vstar_T + 128*sum(khat)).

The program is input-value-independent; built/compiled once, reused.
"""

from contextlib import ExitStack

import numpy as np

import bass_rust
import concourse.bass as bass
import concourse.mybir as mybir
from concourse.bass_utils import run_bass_kernel_spmd

F32 = mybir.dt.float32
BF16 = mybir.dt.bfloat16
AF = mybir.ActivationFunctionType
OP = mybir.AluOpType
NEG = -1e30
EPS = 1e-7

B, T, C, U = 256, 512, 128, 48
S = 2 * U + 1          # 97
BLANK = C - 1
NCORES = 8
BPC = B // NCORES      # 32
NSEG = 4
SEG = T // NSEG        # 128
W = SEG + 1            # cell width (halo slot + 128 values)
NCYC = S + NSEG - 1    # 100
LEAD = 2
KHAT = (0.252, 0.137, 0.137, 0.137)
KSUM = SEG * sum(KHAT)
GRP = 8                # b per mega-DMA
NGRP = BPC // GRP      # 4
PSLAB = NCYC * SEG     # 12800
CH = PSLAB // 4        # hop2 chunk cells
CYC_CH = NCYC // 4     # cycles gated per chunk
VSLAB = (NCYC + LEAD) * W

_cache = {}


def _cb(s0):
    return (s0 + LEAD) * W


def build_program():
    nc = bass.Bass()
    ygpack = nc.declare_dram_parameter("ygpack", [BPC, C, T + S], BF16, isOutput=False)
    mlog = nc.declare_dram_parameter("mlog", [128, NCYC], F32, isOutput=False)
    mlin = nc.declare_dram_parameter("mlin", [128, NCYC], F32, isOutput=False)
    perm = nc.declare_dram_parameter("perm", [128, 128], F32, isOutput=False)
    paug = nc.declare_dram_parameter("paug", [128, 128], F32, isOutput=False)
    negc = nc.declare_dram_parameter("negc", [128, 1], F32, isOutput=False)
    loss = nc.declare_dram_parameter("loss", [BPC, 1], F32, isOutput=True)
    scratch = nc.dram_tensor("scratch", [128, PSLAB], F32)

    ctx = ExitStack()

    def sbuf(shape, name, dt=F32):
        return ctx.enter_context(nc.sbuf_tensor(name, shape, dt))

    def psumt(shape, name):
        return ctx.enter_context(nc.psum_tensor(name, shape, F32))

    def semp(name):
        return ctx.enter_context(nc.semaphore(name))

    with ctx:
        permst = sbuf([128, 128], "permst")
        paugt = sbuf([128, 128], "paugt")
        negct = sbuf([128, 1], "negct")
        mlogt = sbuf([128, NCYC], "mlogt")
        mlint = sbuf([128, NCYC], "mlint")
        ygt = [sbuf([C, GRP * (T + S)], f"ygt{i}", BF16) for i in range(2)]
        stg = [sbuf([S, T], f"stg{i}") for i in range(4)]
        pslab = sbuf([128, PSLAB], "pslab")
        phslab = sbuf([128, PSLAB], "phslab")
        vslab = sbuf([128, VSLAB], "vslab")
        uu = [sbuf([128, SEG], f"u{i}") for i in range(2)]
        negs = sbuf([128, 3 * SEG], "negs")
        atile = sbuf([128, 1], "atile")
        ctile = sbuf([128, 1], "ctile")
        btile = sbuf([128, 1], "btile")
        khat_t = sbuf([128, 1], "khat_t")
        d1 = sbuf([128, 1], "d1")
        bias_t = sbuf([128, 1], "bias_t")
        rout = [sbuf([128, 1], f"rout{j}") for j in range(NSEG)]
        vt = sbuf([128, 1], "vt")
        lt = sbuf([128, 1], "lt")
        st = sbuf([128, 1], "st")
        lossT = sbuf([128, 1], "lossT")

        psg = [psumt([S, T], f"psg{i}") for i in range(4)]
        ph = [psumt([128, 1], f"ph{i}") for i in range(2)]
        bps = psumt([128, 1], "bps")

        sem_c = semp("sem_c")
        sem_yg = [semp(f"sem_yg{g}") for g in range(NGRP)]
        sem_h1 = semp("sem_h1")   # hop1 scratch-write DMAs (SP queue)
        sem_k = [semp(f"sem_k{k}") for k in range(4)]  # hop2 chunks
        # chunks 0,2 issue on SP; 1,3 on gpsimd; one sem each so waits
        # stay on race-detector-valid boundaries
        sem_v = semp("sem_v")
        sem_a = semp("sem_a")
        sem_p = semp("sem_p")
        sem_o = semp("sem_o")
        sem_n = semp("sem_n")     # negs strip ready (DVE)
        sem_m = semp("sem_m")     # scratch margin-fill DMAs (SP)

        # ---- planned semaphore tick values ----
        # PE: 32 gather mms (1..32), viterbi perms (33..131), btile perm
        # (132), linear perms (133..231)
        p_mm = {b: b + 1 for b in range(BPC)}
        p_perm_v = {s0: BPC + 1 + s0 for s0 in range(NCYC - 1)}
        p_bperm = BPC + NCYC
        p_perm_l = {s0: p_bperm + 1 + s0 for s0 in range(NCYC - 1)}
        # ACT: viterbi halos (1..99), atile/ctile copies (-> 106), btile
        # (107), exps (108..111), linear halos (112..210), Ln (211),
        # final (212)
        a_hv = {s0: 1 + s0 for s0 in range(NCYC - 1)}
        a_abc = NCYC - 1 + 7
        a_btile = a_abc + 1
        a_exp = {k: a_btile + 1 + k for k in range(4)}
        a_hl = {s0: a_exp[3] + 1 + s0 for s0 in range(NCYC - 1)}
        a_ln = a_hl[NCYC - 2] + 1
        a_fin = a_ln + 1
        # DVE: psg->stg copies (1..32), viterbi scans (33..132), reduces
        # (133..136), d1 (137), bias (138), linear scans (139..238), vt
        # (239), st (240)
        v_cp = {b: b + 1 for b in range(BPC)}
        v_scan_v = {s0: BPC + 1 + s0 for s0 in range(NCYC)}
        v_red = {j: BPC + NCYC + 1 + j for j in range(NSEG)}
        v_bias = BPC + NCYC + NSEG + 2
        v_scan_l = {s0: v_bias + 1 + s0 for s0 in range(NCYC)}
        v_vt = v_bias + NCYC + 1
        v_st = v_vt + 1

        def hop1_out(b):
            o = scratch[:]
            o.ap = bass_rust.VecI64Pair(
                [[SEG, S], [32 * PSLAB + SEG, NSEG], [1, SEG]])
            o.offset = b * PSLAB
            return o

        with nc.Block() as block:

            def chunk_dma(q, k, sem):
                q.dma_start(
                    pslab[:, k * CH:(k + 1) * CH],
                    scratch[:, k * CH:(k + 1) * CH],
                ).then_inc(sem, 16)

            @block.sync
            def _(sync):
                sync.dma_start(permst[:], perm[:]).then_inc(sem_c, 16)
                sync.dma_start(paugt[:], paug[:]).then_inc(sem_c, 16)
                sync.dma_start(negct[:], negc[:]).then_inc(sem_c, 16)
                sync.dma_start(mlogt[:], mlog[:]).then_inc(sem_c, 16)
                sync.dma_start(mlint[:], mlin[:]).then_inc(sem_c, 16)
                # pre-fill the skew-margin cells of scratch with NEG; the
                # hop1 writes below overwrite the valid cells
                sync.wait_ge(sem_n, 1)
                sync.dma_start(scratch[:, 0:3 * SEG],
                               negs[:]).then_inc(sem_m, 16)
                sync.dma_start(scratch[:, PSLAB - 3 * SEG:PSLAB],
                               negs[:]).then_inc(sem_m, 16)
                for b in range(BPC):
                    sync.wait_ge(sem_v, v_cp[b])
                    sync.dma_start(
                        hop1_out(b),
                        stg[b % 4][:].rearrange("s (j w) -> s j w", w=SEG),
                    ).then_inc(sem_h1, 16)
                sync.wait_ge(sem_h1, 16 * BPC)
                chunk_dma(sync, 0, sem_k[0])
                chunk_dma(sync, 2, sem_k[2])
                sync.wait_ge(sem_a, a_fin)
                sync.dma_start(loss[:, :], lossT[96:128, :]).then_inc(sem_o, 16)
                sync.wait_ge(sem_o, 16)

            @block.gpsimd
            def _(gpsimd):
                ygr = ygpack[:].rearrange("b c w -> c b w")
                for g in range(NGRP):
                    if g >= 2:
                        gpsimd.wait_ge(sem_p, p_mm[(g - 1) * GRP - 1])
                    gpsimd.dma_start(
                        ygt[g % 2][:].rearrange("c (b w) -> c b w", w=T + S),
                        ygr[:, g * GRP:(g + 1) * GRP, :],
                    ).then_inc(sem_yg[g], 16)
                gpsimd.wait_ge(sem_h1, 16 * BPC)
                chunk_dma(gpsimd, 1, sem_k[1])
                chunk_dma(gpsimd, 3, sem_k[3])

            @block.tensor
            def _(tensor):
                for b in range(BPC):
                    g = b // GRP
                    if b % GRP == 0:
                        tensor.wait_ge(sem_yg[g], 16)
                    if b >= 4:
                        tensor.wait_ge(sem_v, v_cp[b - 4])
                    yg3 = ygt[g % 2][:].rearrange("c (b w) -> c b w", w=T + S)
                    bl = b % GRP
                    nc.tensor.matmul(
                        psg[b % 4][:], lhsT=yg3[:, bl, T:T + S],
                        rhs=yg3[:, bl, 0:T], start=True, stop=True,
                    ).then_inc(sem_p, 1)

                def perms(v_scan, a_h, aug):
                    for s0 in range(NCYC - 1):
                        tensor.wait_ge(sem_v, v_scan[s0])
                        if s0 >= 2:
                            tensor.wait_ge(sem_a, a_h[s0 - 2])
                        if aug:
                            nc.tensor.matmul(
                                ph[s0 % 2][:], lhsT=permst[:],
                                rhs=vslab[:, _cb(s0) + SEG:_cb(s0) + SEG + 1],
                                start=True, stop=False,
                            )
                            nc.tensor.matmul(
                                ph[s0 % 2][:], lhsT=paugt[:], rhs=negct[:],
                                start=False, stop=True,
                            ).then_inc(sem_p, 1)
                        else:
                            nc.tensor.matmul(
                                ph[s0 % 2][:], lhsT=permst[:],
                                rhs=vslab[:, _cb(s0) + SEG:_cb(s0) + SEG + 1],
                                start=True, stop=True,
                            ).then_inc(sem_p, 1)

                tensor.wait_ge(sem_c, 80)
                perms(v_scan_v, a_hv, True)
                tensor.wait_ge(sem_a, a_abc)
                nc.tensor.matmul(bps[:], lhsT=permst[:], rhs=ctile[:],
                                 start=True, stop=True).then_inc(sem_p, 1)
                perms(v_scan_l, a_hl, False)

            @block.scalar
            def _(scalar):
                def halos(p_perm):
                    for s0 in range(NCYC - 1):
                        scalar.wait_ge(sem_p, p_perm[s0])
                        nc.scalar.activation(
                            out=vslab[32:64, _cb(s0 + 1):_cb(s0 + 1) + 1],
                            in_=ph[s0 % 2][32:64], func=AF.Copy)
                        nc.scalar.activation(
                            out=vslab[64:128, _cb(s0 + 1):_cb(s0 + 1) + 1],
                            in_=ph[s0 % 2][64:128], func=AF.Copy,
                        ).then_inc(sem_a, 1)

                halos(p_perm_v)
                for j in range(1, NSEG + 1):
                    scalar.wait_ge(sem_v, v_red[j - 1])
                    lo, hi = 32 * (j - 1), 32 * j
                    nc.scalar.activation(out=atile[lo:hi], in_=rout[j - 1][lo:hi],
                                         func=AF.Copy).then_inc(sem_a, 1)
                    if j < NSEG:
                        nc.scalar.activation(out=ctile[lo:hi],
                                             in_=rout[j - 1][lo:hi],
                                             func=AF.Copy).then_inc(sem_a, 1)
                scalar.wait_ge(sem_p, p_bperm)
                nc.scalar.activation(out=btile[:], in_=bps[:],
                                     func=AF.Copy).then_inc(sem_a, 1)
                scalar.wait_ge(sem_v, v_bias)
                for k in range(4):
                    nc.scalar.activation(
                        out=phslab[:, k * CH:(k + 1) * CH],
                        in_=pslab[:, k * CH:(k + 1) * CH], func=AF.Exp,
                        bias=bias_t[:], scale=1.0).then_inc(sem_a, 1)
                halos(p_perm_l)
                scalar.wait_ge(sem_v, v_vt)
                nc.scalar.activation(out=lt[96:128], in_=vt[96:128],
                                     func=AF.Ln).then_inc(sem_a, 1)
                scalar.wait_ge(sem_v, v_st)
                nc.scalar.activation(out=lossT[96:128], in_=st[96:128],
                                     func=AF.Copy, scale=-1.0,
                                     bias=-KSUM).then_inc(sem_a, 1)

            @block.vector
            def _(vector):
                v3 = vslab[:].rearrange("p (c w) -> p c w", w=W)

                def init_slab(viterbi):
                    z = NEG if viterbi else 0.0
                    nc.vector.memset(vslab[:, 0:LEAD * W], z)
                    nc.vector.memset(v3[:, LEAD:, 0], z)
                    vector.drain()
                    nc.vector.memset(vslab[0:32, _cb(0):_cb(0) + 1],
                                     0.0 if viterbi else 1.0)

                init_slab(True)
                nc.vector.memset(negs[:], NEG).then_inc(sem_n, 1)
                vector.wait_ge(sem_m, 32)
                for b in range(BPC):
                    vector.wait_ge(sem_p, p_mm[b])
                    if b >= 4 and b % 4 == 0:
                        # prior stg slot group fully flushed to scratch
                        vector.wait_ge(sem_h1, 64 * (b // 4))
                    nc.vector.tensor_scalar_add(
                        stg[b % 4][:], psg[b % 4][:], 0.0).then_inc(sem_v, 1)
                vector.wait_ge(sem_c, 80)

                def cycles(viterbi, data_slab, a_h, p_perm):
                    for s0 in range(NCYC):
                        if viterbi and s0 % CYC_CH == 0:
                            vector.wait_ge(sem_k[s0 // CYC_CH], 16)
                        if not viterbi and s0 % CYC_CH == 0:
                            vector.wait_ge(sem_a, a_exp[s0 // CYC_CH])
                        if s0 >= 2:
                            vector.wait_ge(sem_a, a_h[s0 - 2])
                        vector.drain()
                        nc.vector.scalar_tensor_tensor(
                            out=uu[s0 % 2][:],
                            in0=vslab[:, _cb(s0 - 2):_cb(s0 - 2) + SEG],
                            scalar=(mlogt if viterbi else mlint)[:, s0:s0 + 1],
                            in1=vslab[:, _cb(s0 - 1):_cb(s0 - 1) + SEG],
                            op0=OP.add if viterbi else OP.mult,
                            op1=OP.max if viterbi else OP.add,
                        )
                        if s0 >= 1:
                            vector.wait_ge(sem_p, p_perm[s0 - 1])
                        vector.drain()
                        nc.vector.tensor_tensor_scan(
                            out=vslab[:, _cb(s0) + 1:_cb(s0) + 1 + SEG],
                            data0=uu[s0 % 2][:],
                            data1=data_slab[:, s0 * SEG:(s0 + 1) * SEG],
                            initial=(ph[(s0 - 1) % 2][:, 0:1] if s0 >= 1
                                     else vslab[:, _cb(s0):_cb(s0) + 1]),
                            op0=OP.max if viterbi else OP.add,
                            op1=OP.add if viterbi else OP.mult,
                        ).then_inc(sem_v, 1)

                cycles(True, pslab, a_hv, p_perm_v)
                vector.drain()
                nc.vector.memset(ctile[:], 0.0)
                for j in range(1, NSEG + 1):
                    nc.vector.tensor_reduce(
                        out=rout[j - 1][:],
                        in_=v3[:, (j - 1) + LEAD:(j - 1) + LEAD + S, SEG],
                        axis=mybir.AxisListType.X, op=OP.max,
                    ).then_inc(sem_v, 1)
                for j in range(NSEG):
                    nc.vector.memset(khat_t[32 * j:32 * (j + 1)], KHAT[j])
                vector.wait_ge(sem_a, a_btile)
                nc.vector.tensor_tensor(out=d1[:], in0=atile[:], in1=btile[:],
                                        op=OP.subtract).then_inc(sem_v, 1)
                vector.drain()
                nc.vector.scalar_tensor_tensor(
                    out=bias_t[:], in0=d1[:], scalar=-1.0 / SEG, in1=khat_t[:],
                    op0=OP.mult, op1=OP.subtract).then_inc(sem_v, 1)
                # linear init: wait until viterbi halo writes to vslab done
                vector.wait_ge(sem_a, a_exp[0])
                init_slab(False)
                cycles(False, phslab, a_hl, p_perm_l)
                vector.drain()
                nc.vector.tensor_tensor(
                    out=vt[96:128],
                    in0=vslab[96:128, _cb(S + 1) + SEG:_cb(S + 1) + SEG + 1],
                    in1=vslab[96:128, _cb(S + 2) + SEG:_cb(S + 2) + SEG + 1],
                    op=OP.add).then_inc(sem_v, 1)
                vector.wait_ge(sem_a, a_ln)
                nc.vector.tensor_tensor(out=st[96:128], in0=lt[96:128],
                                        in1=atile[96:128],
                                        op=OP.add).then_inc(sem_v, 1)

    return nc


def host_prep(y_true, y_pred):
    import ml_dtypes
    y_true = np.asarray(y_true)
    y_pred = np.asarray(y_pred, dtype=np.float32)
    ext = np.full((B, S), BLANK, dtype=np.int64)
    ext[:, 1::2] = y_true.astype(np.int64)
    sh = np.concatenate([np.full((B, 2), -1, dtype=np.int64), ext[:, :-2]], axis=1)
    m = ((ext != BLANK) & (ext != sh))

    lq = np.log(y_pred + EPS).astype(np.float32)  # [B, T, C]

    in_maps = []
    for k in range(NCORES):
        bs = slice(k * BPC, (k + 1) * BPC)
        lqt = np.transpose(lq[bs], (0, 2, 1))  # [32, C, T]
        g = np.zeros((BPC, C, S), dtype=np.float32)
        eb = ext[bs]
        for b in range(BPC):
            g[b, eb[b], np.arange(S)] = 1.0
        ygp = np.ascontiguousarray(
            np.concatenate([lqt, g], axis=2)).astype(ml_dtypes.bfloat16)
        mk = m[bs]
        mlogv = np.full((128, NCYC), NEG, dtype=np.float32)
        mlinv = np.zeros((128, NCYC), dtype=np.float32)
        for j in range(NSEG):
            for s0 in range(NCYC):
                s = s0 - j
                if 0 <= s < S:
                    mlogv[32 * j:32 * (j + 1), s0] = np.where(mk[:, s], 0.0, NEG)
                    mlinv[32 * j:32 * (j + 1), s0] = mk[:, s].astype(np.float32)
        permv = np.zeros((128, 128), dtype=np.float32)
        for kk in range(96):
            permv[kk, kk + 32] = 1.0
        paugv = np.zeros((128, 128), dtype=np.float32)
        for kk in range(32):
            paugv[kk, kk] = 1.0
        negcv = np.full((128, 1), NEG, dtype=np.float32)
        in_maps.append({"ygpack": ygp, "mlog": mlogv, "mlin": mlinv,
                        "perm": permv, "paug": paugv, "negc": negcv})
    return in_maps


def _ensure_axon_devices():
    """Best-effort: make sure the axon PJRT devices are visible even if the
    calling process pinned jax_platforms to cpu (the reference needs cpu;
    run_bass_kernel_spmd needs the 8 NeuronCore devices)."""
    import jax
    try:
        devs = jax.devices()
        if len(devs) >= NCORES and all(d.platform != "cpu" for d in devs[:1]):
            return
    except Exception:
        pass
    try:
        jax.config.update("jax_platforms", None)
        jax.devices()
    except Exception:
        pass


def kernel(y_true, y_pred):
    _ensure_axon_devices()
    if "nc" not in _cache:
        _cache["nc"] = build_program()
    nc = _cache["nc"]
    in_maps = host_prep(y_true, y_pred)
    res = run_bass_kernel_spmd(nc, in_maps, list(range(NCORES)))
    out = np.concatenate([np.asarray(res.results[k]["loss"], dtype=np.float32)
                          for k in range(NCORES)], axis=0)
    return out.reshape(B, 1).astype(np.float32)
